# revision 1
# baseline (speedup 1.0000x reference)
"""Dilated 3-layer LSTM (DRNN) Trainium2 Bass kernel.

Problem: x [128, 1024, 128] f32 -> y [128, 1024, 256] f32.
Layer l has dilation d in [1, 2, 4]: at step t the layer updates only when
t % d == 0, with input = (x_t for l=0, h_{l-1}(t) otherwise). Output is h2
after each step (piecewise-constant over blocks of 4 steps).

Strategy (8 NeuronCores, data parallel over batch, B_local = 16 per core):
Three sequential phases, one per layer. Layer l is an ordinary LSTM over its
T/d update steps once its input-side projection is precomputed. Per phase:
the input projection (x @ W_ih0.T for l=0, h_{l-1} @ W_ihl.T otherwise) is
computed in 16-step blocks as a high-utilization GEMM straight into PSUM;
the recurrent scan then accumulates W_hh @ h_{t-1} on top (start=False) and
runs the LSTM cell in a transposed gate layout [128 partitions = gate dim,
free = (chunk, batch)] so ACT/DVE ops are short in the free dimension.

Weights live in SBUF as pre-transposed bf16 [K=128, M=128] matmul tiles; all
hidden-state history is kept in SBUF (bf16) and doubles as the next layer's
GEMM input. Only x is read from and y written to DRAM.
"""

import numpy as np
import ml_dtypes

import concourse.bass as bass
import concourse.bacc as bacc
import concourse.mybir as mybir
import concourse.tile as tile
from concourse.bass_utils import run_bass_kernel_spmd

F32 = mybir.dt.float32
BF16 = mybir.dt.bfloat16
SIGMOID = mybir.ActivationFunctionType.Sigmoid
TANH = mybir.ActivationFunctionType.Tanh
MULT = mybir.AluOpType.mult
ADD = mybir.AluOpType.add

N_CORES = 8
B_FULL, T_FULL, F_IN, H = 128, 1024, 128, 256
B = B_FULL // N_CORES  # 16 per core
S = 32  # scan steps per GEMM block (one PSUM bank per m-chunk)
# m-chunk order within the 8 x 128 gate rows: [i0,i1,f0,f1,o0,o1,g0,g1]
# (PyTorch LSTM rows are i,f,g,o; we place o before g so sigmoid covers a
# contiguous m-range 0:6 and tanh covers 6:8).
M_ORDER = [0, 1, 2, 3, 6, 7, 4, 5]

_NWAIT_PATCHED = False


def _install_drain_patch():
    """The AWS walrus in this env rejects instructions carrying many sem
    waits (the TileContext final drain aggregates one per logical proc).
    Split those waits across single-wait NOPs on the sync engine."""
    global _NWAIT_PATCHED
    if _NWAIT_PATCHED:
        return
    _NWAIT_PATCHED = True
    import concourse.tile as tile_mod
    from concourse.vector_clock import ScopedClock
    from bass_rust import VectorClock

    def _split_drain_and_barrier(self, tick_clock, wait_clock):
        gc = tick_clock.global_clock
        n = len(gc)
        procs = [(i, gc[i]) for i in range(n) if gc[i] > 0]
        for i, t in procs:
            sub = VectorClock([0] * n)
            sub.require_at_least(i, t)
            d = self.nc.sync.nop(nofuse=True, hint="drain_split_wait")
            wait_clock.add_sem_waits(d.ins, ScopedClock({None: sub}))
        self.nc.sync.drain()
        self.nc.all_engine_barrier()
        popped = self.nc._tile_sem_poison_stack.pop()
        assert popped is self._sem_poison
        self.nc.clear_and_free_semaphores(list(self.sems.allocated().values()))
        self.nc.all_engine_barrier()

    tile_mod.TileContext._drain_and_barrier = _split_drain_and_barrier


# ----------------------------------------------------------------------------
# host-side input prep
# ----------------------------------------------------------------------------

def _prep_w(w: np.ndarray, dtype=None) -> np.ndarray:
    """[4H, in_dim] f32 -> [128, kc, 8, 128] (bf16 by default) of
    pre-transposed lhsT tiles: out[:, k, mi, :] = W[rows(mi), kk].T"""
    in_dim = w.shape[1]
    kc = in_dim // 128
    out = np.empty((128, kc, 8, 128), dtype=dtype or ml_dtypes.bfloat16)
    for mi, rc in enumerate(M_ORDER):
        blk = w[rc * 128:(rc + 1) * 128, :]  # [M=128, in_dim]
        for k in range(kc):
            out[:, k, mi, :] = blk[:, k * 128:(k + 1) * 128].T
    return out


def _prep_b(b_ih: np.ndarray, b_hh: np.ndarray) -> np.ndarray:
    b = (b_ih + b_hh).astype(np.float32)
    out = np.empty((1, 8, 128), dtype=np.float32)
    for mi, rc in enumerate(M_ORDER):
        out[0, mi, :] = b[rc * 128:(rc + 1) * 128]
    return out


def _prep_core_inputs(inputs: dict, T: int) -> list[dict]:
    x = np.asarray(inputs["x"], dtype=np.float32)
    shared = {}
    shared["wih0"] = _prep_w(np.asarray(inputs["W_ih0"]))
    shared["whh0"] = _prep_w(np.asarray(inputs["W_hh0"]))
    shared["wih1"] = _prep_w(np.asarray(inputs["W_ih1"]))
    shared["whh1"] = _prep_w(np.asarray(inputs["W_hh1"]))
    shared["wih2"] = _prep_w(np.asarray(inputs["W_ih2"]))
    shared["whh2"] = _prep_w(np.asarray(inputs["W_hh2"]))
    shared["bias0"] = _prep_b(inputs["b_ih0"], inputs["b_hh0"])
    shared["bias1"] = _prep_b(inputs["b_ih1"], inputs["b_hh1"])
    shared["bias2"] = _prep_b(inputs["b_ih2"], inputs["b_hh2"])
    in_maps = []
    for c in range(N_CORES):
        xs = x[c * B:(c + 1) * B, :T, :]             # [B, T, F]
        xT = np.ascontiguousarray(xs.transpose(2, 1, 0)).reshape(F_IN, T * B)
        m = dict(shared)
        m["xT"] = xT.astype(ml_dtypes.bfloat16)
        in_maps.append(m)
    return in_maps


# ----------------------------------------------------------------------------
# device program
# ----------------------------------------------------------------------------

def build_nc(T: int = T_FULL, debug: bool = False, emit_y: bool = True):
    assert T % 4 == 0 and (T // 4) % 8 == 0
    nc = bacc.Bacc()

    xT = nc.declare_dram_parameter("xT", [F_IN, T * B], BF16, isOutput=False)
    wih0 = nc.declare_dram_parameter("wih0", [128, 1, 8, 128], BF16, isOutput=False)
    whh0 = nc.declare_dram_parameter("whh0", [128, 2, 8, 128], BF16, isOutput=False)
    wih1 = nc.declare_dram_parameter("wih1", [128, 2, 8, 128], BF16, isOutput=False)
    whh1 = nc.declare_dram_parameter("whh1", [128, 2, 8, 128], BF16, isOutput=False)
    wih2 = nc.declare_dram_parameter("wih2", [128, 2, 8, 128], BF16, isOutput=False)
    whh2 = nc.declare_dram_parameter("whh2", [128, 2, 8, 128], BF16, isOutput=False)
    bias0 = nc.declare_dram_parameter("bias0", [1, 8, 128], F32, isOutput=False)
    bias1 = nc.declare_dram_parameter("bias1", [1, 8, 128], F32, isOutput=False)
    bias2 = nc.declare_dram_parameter("bias2", [1, 8, 128], F32, isOutput=False)
    y = nc.declare_dram_parameter("y", [B, T, H], F32, isOutput=True)
    if debug:
        h0d = nc.declare_dram_parameter("h0d", [128, T + 1, 2, B], BF16, isOutput=True)
        h1d = nc.declare_dram_parameter("h1d", [128, T // 2 + 1, 2, B], BF16, isOutput=True)
        h2d = nc.declare_dram_parameter("h2d", [128, T // 4 + 1, 2, B], BF16, isOutput=True)

    with tile.TileContext(nc) as tc:
        with (
            tc.tile_pool(name="const", bufs=1) as cpool,
            tc.tile_pool(name="state", bufs=1) as spool,
            tc.tile_pool(name="xb", bufs=3) as xpool,
            tc.tile_pool(name="cell", bufs=8) as cellpool,
            tc.tile_pool(name="ystg", bufs=3) as ypool,
        ):
            # -- persistent constants ----------------------------------------
            w_ih0 = cpool.tile([128, 1, 8, 128], BF16, tag="wih0")
            w_hh0 = cpool.tile([128, 2, 8, 128], BF16, tag="whh0")
            w_ih1 = cpool.tile([128, 2, 8, 128], BF16, tag="wih1")
            w_hh1 = cpool.tile([128, 2, 8, 128], BF16, tag="whh1")
            w_ih2 = cpool.tile([128, 2, 8, 128], BF16, tag="wih2")
            w_hh2 = cpool.tile([128, 2, 8, 128], BF16, tag="whh2")
            b0 = cpool.tile([1, 8, 128], F32, tag="b0")
            b1 = cpool.tile([1, 8, 128], F32, tag="b1")
            b2 = cpool.tile([1, 8, 128], F32, tag="b2")
            ones = cpool.tile([1, S * B], F32, tag="ones")
            for sb, dr in ((w_ih0, wih0), (w_hh0, whh0), (w_ih1, wih1),
                           (w_hh1, whh1), (w_ih2, wih2), (w_hh2, whh2),
                           (b0, bias0), (b1, bias1), (b2, bias2)):
                nc.sync.dma_start(sb[:], dr[:])
            nc.vector.memset(ones[:], 1.0)

            # -- state buffers: slot t+1 = h after step t; slot 0 = zeros ----
            H0 = spool.tile([128, T + 1, 2, B], BF16, tag="H0")
            H1 = spool.tile([128, T // 2 + 1, 2, B], BF16, tag="H1")
            H2 = spool.tile([128, T // 4 + 1, 2, B], BF16, tag="H2")
            cc = [spool.tile([128, 4, B], F32, tag=f"cc{i}", name=f"cc{i}")
                  for i in range(3)]
            for hb in (H0, H1, H2):
                nc.vector.memset(hb[:, 0, :, :], 0.0)
            for c in cc:
                nc.vector.memset(c[:, 2:4, :], 0.0)

            def cell(gb, s, ccl, h_out, hf_out=None):
                """LSTM cell in transposed layout; gates in gb[:, :, s*B:(s+1)*B]."""
                sg = cellpool.tile([128, 6, B], F32, tag="sig")
                vp = cellpool.tile([128, 4, B], F32, tag="vp")
                tct = cellpool.tile([128, 2, B], F32, tag="tct")
                gs = gb[:, :, s * B:(s + 1) * B]
                nc.scalar.activation(sg[:], gs[:, 0:6, :], SIGMOID)
                nc.scalar.activation(ccl[:, 0:2, :], gs[:, 6:8, :], TANH)
                nc.vector.tensor_tensor(vp[:], sg[:, 0:4, :], ccl[:, 0:4, :], MULT)
                nc.vector.tensor_tensor(ccl[:, 2:4, :], vp[:, 0:2, :], vp[:, 2:4, :], ADD)
                nc.scalar.activation(tct[:], ccl[:, 2:4, :], TANH)
                if hf_out is None:
                    nc.vector.tensor_tensor(h_out, sg[:, 4:6, :], tct[:], MULT)
                else:
                    nc.vector.tensor_tensor(hf_out[:], sg[:, 4:6, :], tct[:], MULT)
                    nc.vector.tensor_copy(h_out, hf_out[:])

            def gemm_block(gb, w_sb, kc, rhs_fn, b_sb, sblk):
                for k in range(kc):
                    for m in range(8):
                        nc.tensor.matmul(gb[:, m, :], w_sb[:, k, m, :], rhs_fn(k),
                                         start=(k == 0), stop=False)
                for m in range(8):
                    nc.tensor.matmul(gb[:, m, :], b_sb[:, m, :],
                                     ones[:, 0:sblk * B],
                                     start=False, stop=True)

            def scan_mm(gb, s, w_sb, h_prev):
                for k in range(2):
                    for m in range(8):
                        nc.tensor.matmul(gb[:, m, s * B:(s + 1) * B],
                                         w_sb[:, k, m, :], h_prev[:, k, :],
                                         start=False, stop=(k == 1),
                                         skip_group_check=True)

            # ================= phase 0: layer 0 =============================
            S0 = min(S, T)
            with tc.tile_pool(name="ps0", bufs=1, space="PSUM") as pp0:
                for blk in range(T // S0):
                    gb = pp0.tile([128, 8, 512], F32, tag="gb", name="gb0")[:, :, 0:S0 * B]
                    xb = xpool.tile([128, S0 * B], BF16, tag="xb")
                    nc.sync.dma_start(xb[:], xT[:, blk * S0 * B:(blk + 1) * S0 * B])
                    gemm_block(gb, w_ih0, 1, lambda k: xb[:], b0, S0)
                    for s in range(S0):
                        t = blk * S0 + s
                        scan_mm(gb, s, w_hh0, H0[:, t, :, :])
                        cell(gb, s, cc[0], H0[:, t + 1, :, :])

            # ================= phase 1: layer 1 =============================
            T1 = T // 2
            S1 = min(S, T1)
            with tc.tile_pool(name="ps1", bufs=1, space="PSUM") as pp1:
                for blk in range(T1 // S1):
                    gb = pp1.tile([128, 8, 512], F32, tag="gb", name="gb1")[:, :, 0:S1 * B]
                    t0 = blk * S1
                    gemm_block(gb, w_ih1, 2,
                               lambda k: H0[:, 2 * t0 + 1:2 * (t0 + S1):2, k, :],
                               b1, S1)
                    for s in range(S1):
                        tau = t0 + s
                        scan_mm(gb, s, w_hh1, H1[:, tau, :, :])
                        cell(gb, s, cc[1], H1[:, tau + 1, :, :])

            # ================= phase 2: layer 2 + output ====================
            T2 = T // 4
            S2 = min(S, T2)
            with tc.tile_pool(name="ps2", bufs=1, space="PSUM") as pp2:
                for blk in range(T2 // S2):
                    gb = pp2.tile([128, 8, 512], F32, tag="gb", name="gb2")[:, :, 0:S2 * B]
                    t0 = blk * S2
                    gemm_block(gb, w_ih2, 2,
                               lambda k: H1[:, 2 * t0 + 1:2 * (t0 + S2):2, k, :],
                               b2, S2)
                    for s in range(S2):
                        rho = t0 + s
                        scan_mm(gb, s, w_hh2, H2[:, rho, :, :])
                        h2f = cellpool.tile([128, 2, B], F32, tag="h2f")
                        cell(gb, s, cc[2], H2[:, rho + 1, :, :], hf_out=h2f)
                        # y path: transpose h2f [128, 32] -> t32 [32, 128]
                        # (4x DVE 32x32 block transposes), then partition-remap
                        # into ystage [B, 8, 2, 128], bulk DMA out every 8.
                        if not emit_y:
                            continue
                        u = rho % 8
                        if u == 0:
                            ystage = ypool.tile([B, 8, 2, 128], F32, tag="ystage")
                        t32 = cellpool.tile([32, 128], F32, tag="t32")
                        h2v = h2f[:].rearrange("p a b -> p (a b)")
                        for j in range(4):
                            nc.vector.transpose(t32[:, j * 32:(j + 1) * 32],
                                                h2v[j * 32:(j + 1) * 32, :])
                        for c2 in range(2):
                            nc.sync.dma_start(ystage[:, u, c2, :],
                                              t32[c2 * B:(c2 + 1) * B, :])
                        if u == 7:
                            for j in range(4):
                                nc.sync.dma_start(
                                    y[:, 4 * (rho - 7) + j:4 * rho + j + 1:4, :]
                                    .rearrange("b u (c h) -> b u c h", c=2),
                                    ystage[:])
            if debug:
                nc.sync.dma_start(h0d[:], H0[:])
                nc.sync.dma_start(h1d[:], H1[:])
                nc.sync.dma_start(h2d[:], H2[:])
    nc.compile()
    return nc


def build_nc_v2(T: int = T_FULL, debug: bool = False, emit_y: bool = True):
    """Interleaved emission: layers pipelined in super-blocks of 16 L0 steps
    (8 L1 steps lagging one block, 4 L2 steps lagging two). All three layers
    share one [128, 8, 512] PSUM tile per super-block: bank m holds L0 gates
    in cols 0:256, L1 in 256:384, L2 in 384:448. The L0 GEMM's start=True
    clears each bank (bank-wide clear), so it must be emitted first; the L1/
    L2 GEMMs then raw-write (start=False) into the cleared regions."""
    SB = 4                        # L0 steps per super-block
    assert T % SB == 0 and (T // 4) % 8 == 0
    NBLK = T // SB
    nc = bacc.Bacc()

    xT = nc.declare_dram_parameter("xT", [F_IN, T * B], BF16, isOutput=False)
    wih0 = nc.declare_dram_parameter("wih0", [128, 1, 8, 128], BF16, isOutput=False)
    whh0 = nc.declare_dram_parameter("whh0", [128, 2, 8, 128], BF16, isOutput=False)
    wih1 = nc.declare_dram_parameter("wih1", [128, 2, 8, 128], BF16, isOutput=False)
    whh1 = nc.declare_dram_parameter("whh1", [128, 2, 8, 128], BF16, isOutput=False)
    wih2 = nc.declare_dram_parameter("wih2", [128, 2, 8, 128], BF16, isOutput=False)
    whh2 = nc.declare_dram_parameter("whh2", [128, 2, 8, 128], BF16, isOutput=False)
    bias0 = nc.declare_dram_parameter("bias0", [1, 8, 128], F32, isOutput=False)
    bias1 = nc.declare_dram_parameter("bias1", [1, 8, 128], F32, isOutput=False)
    bias2 = nc.declare_dram_parameter("bias2", [1, 8, 128], F32, isOutput=False)
    y = nc.declare_dram_parameter("y", [B, T, H], F32, isOutput=True)
    if debug:
        h0d = nc.declare_dram_parameter("h0d", [128, T + 1, 2, B], BF16, isOutput=True)
        h1d = nc.declare_dram_parameter("h1d", [128, T // 2 + 1, 2, B], BF16, isOutput=True)
        h2d = nc.declare_dram_parameter("h2d", [128, T // 4 + 1, 2, B], BF16, isOutput=True)

    with tile.TileContext(nc) as tc:
        with (
            tc.tile_pool(name="const", bufs=1) as cpool,
            tc.tile_pool(name="state", bufs=1) as spool,
            tc.tile_pool(name="xb", bufs=3) as xpool,
            tc.tile_pool(name="cell", bufs=8) as cellpool,
            tc.tile_pool(name="ystg", bufs=3) as ypool,
            tc.tile_pool(name="ps", bufs=1, space="PSUM") as ppool,
        ):
            w_ih0 = cpool.tile([128, 1, 8, 128], BF16, tag="wih0")
            w_hh0 = cpool.tile([128, 2, 8, 128], BF16, tag="whh0")
            w_ih1 = cpool.tile([128, 2, 8, 128], BF16, tag="wih1")
            w_hh1 = cpool.tile([128, 2, 8, 128], BF16, tag="whh1")
            w_ih2 = cpool.tile([128, 2, 8, 128], BF16, tag="wih2")
            w_hh2 = cpool.tile([128, 2, 8, 128], BF16, tag="whh2")
            b0 = cpool.tile([1, 8, 128], F32, tag="b0")
            b1 = cpool.tile([1, 8, 128], F32, tag="b1")
            b2 = cpool.tile([1, 8, 128], F32, tag="b2")
            ones = cpool.tile([1, 256], F32, tag="ones")
            for sb, dr in ((w_ih0, wih0), (w_hh0, whh0), (w_ih1, wih1),
                           (w_hh1, whh1), (w_ih2, wih2), (w_hh2, whh2),
                           (b0, bias0), (b1, bias1), (b2, bias2)):
                nc.sync.dma_start(sb[:], dr[:])
            nc.vector.memset(ones[:], 1.0)

            H0 = spool.tile([128, T + 1, 2, B], BF16, tag="H0")
            H1 = spool.tile([128, T // 2 + 1, 2, B], BF16, tag="H1")
            H2 = spool.tile([128, T // 4 + 1, 2, B], BF16, tag="H2")
            cc = [spool.tile([128, 4, B], F32, tag=f"cc{i}", name=f"cc{i}")
                  for i in range(3)]
            # h2fp holds layer-2 h in fp32 for the y path (y is fp32)
            h2fp = [spool.tile([128, 2, 2, B], F32, tag="h2fp", name="h2fp")]
            for hb in (H0, H1, H2):
                nc.vector.memset(hb[:, 0, :, :], 0.0)
            for c in cc:
                nc.vector.memset(c[:, 2:4, :], 0.0)

            # column base offsets inside each bank, in units of B columns
            OFF = {0: 0, 1: SB, 2: SB + SB // 2}   # L0: 0, L1: 256/B, L2: 384/B

            def cell(gb, col, ccl, h_out, hf_out=None):
                sg = cellpool.tile([128, 6, B], F32, tag="sig")
                vp = cellpool.tile([128, 4, B], F32, tag="vp")
                tct = cellpool.tile([128, 2, B], F32, tag="tct")
                gs = gb[:, :, col * B:(col + 1) * B]
                nc.scalar.activation(ccl[:, 0:2, :], gs[:, 6:8, :], TANH)
                nc.scalar.activation(sg[:], gs[:, 0:6, :], SIGMOID)
                nc.vector.tensor_tensor(vp[:], sg[:, 0:4, :], ccl[:, 0:4, :], MULT)
                nc.vector.tensor_tensor(ccl[:, 2:4, :], vp[:, 0:2, :], vp[:, 2:4, :], ADD)
                nc.scalar.activation(tct[:], ccl[:, 2:4, :], TANH)
                if hf_out is None:
                    nc.vector.tensor_tensor(h_out, sg[:, 4:6, :], tct[:], MULT)
                else:
                    nc.vector.tensor_tensor(hf_out, sg[:, 4:6, :], tct[:], MULT)
                    nc.vector.tensor_copy(h_out, hf_out)

            def gemm(gb, w_sb, kc, rhs_fn, b_sb, col0, ncols, first):
                lo, hi = col0 * B, (col0 + ncols) * B
                for k in range(kc):
                    for m in range(8):
                        nc.tensor.matmul(gb[:, m, lo:hi], w_sb[:, k, m, :],
                                         rhs_fn(k), start=(first and k == 0),
                                         stop=False, skip_group_check=True)
                for m in range(8):
                    nc.tensor.matmul(gb[:, m, lo:hi], b_sb[:, m, :],
                                     ones[:, 0:ncols * B],
                                     start=False, stop=True,
                                     skip_group_check=True)

            # scan MM emission order puts the g-gate chunks (m=6,7) first so
            # the tanh can start while the remaining chunks stream.
            SCAN_M = [6, 7, 0, 1, 2, 3, 4, 5]

            def scan_mm(gb, col, w_sb, h_prev):
                # m-outer: each gate chunk's two k-matmuls complete adjacently,
                # so tanh(g) starts after 4 matmuls and the o-gate matmuls
                # overlap the start of the cell chain.
                for m in SCAN_M:
                    for k in range(2):
                        nc.tensor.matmul(gb[:, m, col * B:(col + 1) * B],
                                         w_sb[:, k, m, :], h_prev[:, k, :],
                                         start=False, stop=(k == 1),
                                         skip_group_check=True)

            def l0_step(gb, j, t):
                scan_mm(gb, j, w_hh0, H0[:, t, :, :])
                cell(gb, j, cc[0], H0[:, t + 1, :, :])

            def l1_step(gb, jj, tau):
                scan_mm(gb, OFF[1] + jj, w_hh1, H1[:, tau, :, :])
                cell(gb, OFF[1] + jj, cc[1], H1[:, tau + 1, :, :])

            def l2_step(gb, jj, rho):
                scan_mm(gb, OFF[2] + jj, w_hh2, H2[:, rho, :, :])
                h2f = h2fp[0][:, rho % 2, :, :]
                cell(gb, OFF[2] + jj, cc[2], H2[:, rho + 1, :, :], hf_out=h2f)
                if not emit_y:
                    return
                u = rho % 8
                if u == 0:
                    l2_step.ystage = ypool.tile([B, 8, 2, 128], F32, tag="ystage")
                ystage = l2_step.ystage
                t32 = cellpool.tile([32, 128], F32, tag="t32")
                h2v = h2f.rearrange("p a b -> p (a b)")
                for jb in range(4):
                    nc.vector.transpose(t32[:, jb * 32:(jb + 1) * 32],
                                        h2v[jb * 32:(jb + 1) * 32, :])
                for c2 in range(2):
                    nc.sync.dma_start(ystage[:, u, c2, :],
                                      t32[c2 * B:(c2 + 1) * B, :])
                if u == 7:
                    for jb in range(4):
                        nc.sync.dma_start(
                            y[:, 4 * (rho - 7) + jb:4 * rho + jb + 1:4, :]
                            .rearrange("b u (c h) -> b u c h", c=2),
                            ystage[:])

            for n in range(NBLK + 2):
                gb = ppool.tile([128, 8, 512], F32, tag="gb", name="gb")
                has_l0 = n < NBLK
                has_l1 = 1 <= n <= NBLK
                has_l2 = 2 <= n <= NBLK + 1
                if has_l0:
                    xb = xpool.tile([128, SB * B], BF16, tag="xb")
                    nc.sync.dma_start(xb[:], xT[:, n * SB * B:(n + 1) * SB * B])
                    gemm(gb, w_ih0, 1, lambda k: xb[:], b0, 0, SB, first=True)
                else:
                    # tail blocks: still need the bank-clearing start=True.
                    # Reuse the L0 GEMM shape with a dummy rhs (xb of block 0)
                    # — results land in the unused L0 region.
                    xb = xpool.tile([128, SB * B], BF16, tag="xb")
                    nc.sync.dma_start(xb[:], xT[:, 0:SB * B])
                    gemm(gb, w_ih0, 1, lambda k: xb[:], b0, 0, SB, first=True)
                for j in range(SB):
                    if has_l0:
                        t = n * SB + j
                        if j == 1 and has_l1:
                            t0 = (n - 1) * SB // 2
                            gemm(gb, w_ih1, 2,
                                 lambda k: H0[:, 2 * t0 + 1:2 * (t0 + SB // 2):2, k, :],
                                 b1, OFF[1], SB // 2, first=False)
                        if j == 2 and has_l2:
                            r0 = (n - 2) * SB // 4
                            gemm(gb, w_ih2, 2,
                                 lambda k: H1[:, 2 * r0 + 1:2 * (r0 + SB // 4):2, k, :],
                                 b2, OFF[2], SB // 4, first=False)
                        l0_step(gb, j, t)
                    else:
                        if j == 1 and has_l1:
                            t0 = (n - 1) * SB // 2
                            gemm(gb, w_ih1, 2,
                                 lambda k: H0[:, 2 * t0 + 1:2 * (t0 + SB // 2):2, k, :],
                                 b1, OFF[1], SB // 2, first=False)
                        if j == 2 and has_l2:
                            r0 = (n - 2) * SB // 4
                            gemm(gb, w_ih2, 2,
                                 lambda k: H1[:, 2 * r0 + 1:2 * (r0 + SB // 4):2, k, :],
                                 b2, OFF[2], SB // 4, first=False)
                    if j % 2 == 1 and has_l1:
                        l1_step(gb, j // 2, (n - 1) * SB // 2 + j // 2)
                    if j % 4 == 3 and has_l2:
                        l2_step(gb, j // 4, (n - 2) * SB // 4 + j // 4)
            if debug:
                nc.sync.dma_start(h0d[:], H0[:])
                nc.sync.dma_start(h1d[:], H1[:])
                nc.sync.dma_start(h2d[:], H2[:])
    nc.compile()
    return nc


# ----------------------------------------------------------------------------
# public entry point
# ----------------------------------------------------------------------------

_CACHE = {}


def _run(inputs: dict, T: int):
    if T not in _CACHE:
        _CACHE[T] = build_nc_v2(T)
    nc = _CACHE[T]
    in_maps = _prep_core_inputs(inputs, T)
    res = run_bass_kernel_spmd(nc, in_maps, list(range(N_CORES)))
    y = np.concatenate([res.results[c]["y"] for c in range(N_CORES)], axis=0)
    return y


def kernel(**inputs) -> np.ndarray:
    return _run(inputs, T_FULL)


if __name__ == "__main__":
    # quick structural check: build a small-T program
    nc = build_nc(64)
    f = nc.m.functions[0]
    n = sum(len(bb.instructions) for bb in f.blocks)
    print(f"built T=64 program: {n} instructions")

